# revision 42
# baseline (speedup 1.0000x reference)
"""Trainium2 Bass kernel for nn_BinaryNN (binary MLP forward pass).

Strategy (8-core data parallel over the batch):
  - Forward of _binarize_weight / _binary_activation is exactly (x > 0), so all
    hidden activations are 0/1 and layers 2-4 are exact integer matmuls, run as
    fp8e4 DoubleRow (paired k-tiles, both operands 3D [128, 2, .]).
  - concat([x, 1-x]) @ W1b == x @ (W1top - W1bot) + colsum(W1bot): halves K to
    784. x splits into a fp16 hi chunk (785 rows incl. the constant-one row
    carrying colsum, + 16 fp16 lo-residual rides in the 896-pad) and a fp8
    DoubleRow lo chunk (768 rows = resid*2^15, err <= 2^-17): 7 fp16 + 3 DR
    matmuls per f-tile instead of 13 fp16 (the PE-bound hot spot; ~2^-17 x-grid
    keeps the h1->h3 flip cascade at rel_err ~5e-3, gate is 2e-2).
  - LayerNorm(scale=1, bias=0) followed by (.>0) reduces to (a > rowmean(a)).
    Layer 1's mean is affine in x -> host-precomputed, DMA partition-broadcast.
    Layers 2/3 row-sums ride as 3 fp8-exact (<=16) weight columns duplicated
    twice; after an ACT scale, lane-local DVE ops build [fp16-lo; fp16-hi]
    rows so a SINGLE K=6 ones-matmul does the sum + exact hi/lo recombine +
    partition-broadcast in one pass (deferred past the first main m-tile so
    the PE never waits on the ACT/DVE chain). Binarize = one DVE
    tensor_tensor(is_gt) per tile, PSUM -> fp8 SBUF.
  - Feature-major layout [features, rows] on chip: no transposes anywhere on
    device; the host pre-transposes x and transposes the [10, B] result back.
  - Overlap: single 3D-AP DMAs, first-needed-first issue order, layer-4
    emission deferred into the next block's L1 stream (the PE is strictly
    in-order), 6 PSUM accumulator banks.
"""

import sys

if "/opt/trn_rl_repo" not in sys.path:
    sys.path.insert(0, "/opt/trn_rl_repo")

import numpy as np
import ml_dtypes

bf16 = ml_dtypes.bfloat16
fp16 = np.float16
fp8 = ml_dtypes.float8_e4m3
LO_SCALE = 32768.0  # 2**15: fp8 lo chunk of x (resid <= 2^-12 -> [-8, 8])

# fp8 weight matrices pad their free dim so the DoubleRow "two"-step is 16B-aligned
W2PAD, W3PAD, W4PAD = 1040, 528, 16
NSUM = 3  # row-sum ints (<=48) split into 3 fp8-exact (<=16) columns
# the 3 sum columns appear TWICE in the weights: rows 0-2 of the sum psum
# become the fp16-lo residuals, rows 3-5 the fp16-hi parts — all lane-local
# DVE ops — so ONE K=6 ones-matmul does the 3-row sum + hi/lo recombine +
# partition-broadcast in a single pass.
NSUM2 = 2 * NSUM

N_CORES = 8
B_FULL = 32768
P = 128
RB = 512  # rows per block (PSUM bank = 512 fp32)

D_IN = 784
NRIDE = 16  # features 768..784 get exact fp16 lo rides instead of fp8 lo
KH = 896  # hi chunk: 784 + ones row + NRIDE rides = 801, padded to 7*128
KL = 768  # fp8 lo chunk: features 0..767 exactly = 6*128 (3 DoubleRow pairs)
F1, F2, F3, NC_OUT = 2048, 1024, 512, 10


def _ktiles(n):
    return [(k0, min(P, n - k0)) for k0 in range(0, n, P)]


def build_bass(n_blocks):
    import concourse.bass as bass  # noqa: F401
    import concourse.mybir as mybir
    import concourse.tile as tile
    from concourse import bacc

    f32 = mybir.dt.float32
    f16 = mybir.dt.float16
    f8 = mybir.dt.float8e4
    DR = mybir.MatmulPerfMode.DoubleRow
    Copy = mybir.ActivationFunctionType.Copy
    is_gt = mybir.AluOpType.is_gt

    R = n_blocks * RB
    nc = bacc.Bacc("TRN2", target_bir_lowering=False, debug=False, num_devices=N_CORES)

    # x is block-major [blk, partition, ktile, row]: each SBUF partition's
    # slice of a block is one contiguous run -> few large DMA descriptors
    xh_d = nc.dram_tensor("xh", [n_blocks, P, KH // P, RB], f16, kind="ExternalInput")
    xl_d = nc.dram_tensor("xl", [n_blocks, P, KL // P, RB], f8, kind="ExternalInput")
    # w1 likewise partition-contiguous, split [0:128] + 4 chunks of 480 cols
    WCH = (F1 - 128) // 4
    w1h0_d = nc.dram_tensor("w1h0", [P, KH // P, 128], f16, kind="ExternalInput")
    w1hb_d = nc.dram_tensor("w1hb", [4, P, KH // P, WCH], f16, kind="ExternalInput")
    w1l0_d = nc.dram_tensor("w1l0", [P, KL // P, 128], f8, kind="ExternalInput")
    w1lb_d = nc.dram_tensor("w1lb", [4, P, KL // P, WCH], f8, kind="ExternalInput")
    m1_d = nc.dram_tensor("m1", [1, R], f32, kind="ExternalInput")
    w2_d = nc.dram_tensor("w2m", [F1, W2PAD], f8, kind="ExternalInput")
    w3_d = nc.dram_tensor("w3m", [F2, W3PAD], f8, kind="ExternalInput")
    w4_d = nc.dram_tensor("w4m", [F3, W4PAD], f8, kind="ExternalInput")
    out_d = nc.dram_tensor("out", [NC_OUT, R], f32, kind="ExternalOutput")

    kth = _ktiles(KH)  # 7 tiles of 128 (fp16 hi)
    ktl = _ktiles(KL)  # 6 tiles of 128 (fp8 lo, 3 DR pairs)
    kt2 = _ktiles(F1)  # 16
    kt3 = _ktiles(F2)  # 8
    kt4 = _ktiles(F3)  # 4

    with tile.TileContext(nc) as tc:
        with (
            tc.tile_pool(name="wpool", bufs=1) as wpool,
            tc.tile_pool(name="xpool", bufs=2) as xpool,
            tc.tile_pool(name="bpool", bufs=2) as bpool,
            tc.tile_pool(name="mpool", bufs=3) as mpool,
            tc.tile_pool(name="opool", bufs=4) as opool,
            tc.tile_pool(name="apool", bufs=6, space="PSUM") as apool,
            tc.tile_pool(name="spool", bufs=1, space="PSUM") as spool,
            tc.tile_pool(name="cpool", bufs=1, space="PSUM") as cpool,
        ):
            # ---- persistent weights (single 3D-AP DMAs) -----------------
            # DMA transfers drain roughly in issue order: block-0 x first,
            # then w1 column-chunk 0 — the minimal set for the first m-tiles.
            x_tiles = {}

            def load_x(blk):
                th = xpool.tile([P, len(kth), RB], f16, tag="xh")
                tl = xpool.tile([P, len(ktl), RB], f8, tag="xl")
                nc.sync.dma_start(out=th[:, 0:4, :], in_=xh_d[blk, :, 0:4, :])
                nc.sync.dma_start(out=th[:, 4:, :], in_=xh_d[blk, :, 4:, :])
                nc.sync.dma_start(out=tl[:], in_=xl_d[blk, :, :, :])
                x_tiles[blk] = (th, tl)

            # block 0: interleave so the first m-tile's exact deps land first —
            # xh half-A, the narrow 128-col w1 chunks, xh half-B, xl
            w1h_sb = wpool.tile([P, len(kth), F1], f16)
            w1l_sb = wpool.tile([P, len(ktl), F1], f8)
            t0_h = xpool.tile([P, len(kth), RB], f16, tag="xh")
            t0_l = xpool.tile([P, len(ktl), RB], f8, tag="xl")
            # block-0 loads split across THREE hw DMA queues (sync, scalar,
            # gpsimd software-DGE), fine-grained and ordered by first use so
            # the first matmuls start as soon as possible
            nc.sync.dma_start(out=t0_h[:, 0:2, :], in_=xh_d[0, :, 0:2, :])
            nc.gpsimd.dma_start(out=w1h_sb[:, 0:2, 0:128], in_=w1h0_d[:, 0:2, :])
            nc.sync.dma_start(out=t0_h[:, 2:4, :], in_=xh_d[0, :, 2:4, :])
            nc.gpsimd.dma_start(out=w1h_sb[:, 2:, 0:128], in_=w1h0_d[:, 2:, :])
            nc.sync.dma_start(out=t0_h[:, 4:, :], in_=xh_d[0, :, 4:, :])
            nc.scalar.dma_start(out=t0_l[:], in_=xl_d[0, :, :, :])
            nc.gpsimd.dma_start(out=w1l_sb[:, :, 0:128], in_=w1l0_d[:, :, :])
            x_tiles[0] = (t0_h, t0_l)
            # w1 column chunks: first hi chunk rides the lighter scalar queue
            # (sync still owes 0.9MB of x), rest split hi-on-sync/lo-on-scalar
            nc.scalar.dma_start(out=w1h_sb[:, :, 128 : 128 + WCH], in_=w1hb_d[0])
            nc.scalar.dma_start(out=w1l_sb[:, :, 128 : 128 + WCH], in_=w1lb_d[0])
            for c in range(1, 4):
                j0 = 128 + c * WCH
                nc.sync.dma_start(
                    out=w1h_sb[:, :, j0 : j0 + WCH], in_=w1hb_d[c, :, :, :]
                )
                nc.scalar.dma_start(
                    out=w1l_sb[:, :, j0 : j0 + WCH], in_=w1lb_d[c, :, :, :]
                )

            w2_sb = wpool.tile([P, len(kt2), W2PAD], f8)
            nc.sync.dma_start(
                out=w2_sb[:], in_=w2_d[:, :].rearrange("(t p) j -> p t j", p=P)
            )
            w3_sb = wpool.tile([P, len(kt3), W3PAD], f8)
            nc.sync.dma_start(
                out=w3_sb[:], in_=w3_d[:, :].rearrange("(t p) j -> p t j", p=P)
            )
            w4_sb = wpool.tile([P, len(kt4), W4PAD], f8)
            nc.sync.dma_start(
                out=w4_sb[:], in_=w4_d[:, :].rearrange("(t p) j -> p t j", p=P)
            )
            ones_sb = wpool.tile([NSUM2, P], f16)
            nc.vector.memset(ones_sb[:], 1.0)



            def mean_bcast_sum(sum_emit, scale, bias):
                """row-sum matmuls -> [6, RB] (3 sum splits, duplicated),
                scaled on ACT, then rows 0-2 become fp16-lo residuals and
                rows 3-5 fp16-hi parts (lane-local DVE), so a single K=6
                ones-matmul sums + broadcasts exactly in one pass.

                Returns (emit_bcast, m_sb): emit_bcast() issues the PE
                matmul and is deferred by the caller until after the first
                main m-tile so the PE never waits on the ACT/DVE chain."""
                sum_ps = spool.tile([NSUM2, RB], f32, tag="sum")
                sum_emit(sum_ps[:])
                m_row = mpool.tile([NSUM2, RB], f32, tag="m_row")
                nc.scalar.activation(m_row[:], sum_ps[:], Copy, bias=bias, scale=scale)
                # engine APs must start at partition 0/32/64/96, so build
                # [lo(0:3); hi(3:6)] with base-0 writes only: tmp = [f16(row);
                # 0] and one 6-row sub (rows 3-5 become f16(row) - 0 = hi)
                m6 = mpool.tile([NSUM2, RB], f16, tag="m6")
                tmp = mpool.tile([NSUM2, RB], f16, tag="m_tmp")
                nc.vector.memset(tmp[:], 0.0)
                nc.vector.tensor_copy(tmp[0:NSUM, :], m_row[0:NSUM, :])
                nc.vector.tensor_sub(m6[:], m_row[:], tmp[:])
                m_sb = mpool.tile([P, RB], f32, tag="m_sb")

                def emit_bcast():
                    m_ps = cpool.tile([P, RB], f32, tag="bcast")
                    nc.tensor.matmul(
                        m_ps[:], ones_sb[:], m6[:], start=True, stop=True
                    )
                    nc.scalar.copy(m_sb[:], m_ps[:])

                return emit_bcast, m_sb

            def norm_binarize(mean_emit, n_mt, mm_emit, sink):
                emit_bcast, m_sb = mean_emit()
                for m in range(n_mt):
                    acc = apool.tile([P, RB], f32, tag="acc")
                    mm_emit(m, acc)
                    if m == 0 and emit_bcast is not None:
                        emit_bcast()  # after m0's mains: ACT/DVE chain hidden
                    sink(m, acc, m_sb)

            def emit_dr(b_tile, w_sb, n_kt, cols, start=True, stop=True):
                """DoubleRow fp8: pairs of k-tiles contracted per matmul."""

                def emit(ps):
                    npair = n_kt // 2
                    for i in range(npair):
                        nc.tensor.matmul(
                            ps,
                            w_sb[:, 2 * i : 2 * i + 2, cols[0] : cols[0] + cols[1]],
                            b_tile[:, 2 * i : 2 * i + 2, :],
                            start=start and (i == 0),
                            stop=stop and (i == npair - 1),
                            perf_mode=DR,
                        )

                return emit

            pending_l4 = [None]  # deferred layer-4 emission (SW pipelining)

            for blk in range(n_blocks):
                c0 = blk * RB
                if blk not in x_tiles:
                    load_x(blk)
                xh_t, xl_t = x_tiles.pop(blk)

                # layer-1 row-mean: affine in x, host-precomputed; partition-
                # broadcast on the idle GpSimd ring so it never queues behind
                # the bulk x/w transfers on the sync ring
                m_sb1 = mpool.tile([P, RB], f32, tag="m_sb")
                _mbase = m1_d[0, c0 : c0 + RB]
                nc.gpsimd.dma_start(
                    out=m_sb1[:],
                    in_=bass.AP(
                        tensor=_mbase.tensor,
                        offset=_mbase.offset,
                        ap=[[0, P]] + list(_mbase.ap),
                    ),
                )

                if blk + 1 < n_blocks:
                    load_x(blk + 1)  # prefetch next block's x

                b1 = bpool.tile([P, len(kt2), RB], f8, tag="b1")

                def sink1(m, acc, m_sb):
                    nc.vector.tensor_tensor(b1[:, m, :], acc[:], m_sb[:], is_gt)

                def mm1(m, acc):
                    # 7 fp16 hi matmuls + 3 fp8 DoubleRow lo matmuls
                    for k in range(len(kth)):
                        nc.tensor.matmul(
                            acc[:],
                            w1h_sb[0:P, k, m * P : m * P + P],
                            xh_t[0:P, k, :],
                            start=(k == 0),
                            stop=False,
                        )
                    emit_dr(xl_t, w1l_sb, len(ktl), (m * P, P), start=False)(acc[:])
                    if m == 1 and pending_l4[0] is not None:
                        # previous block's L4: its b3 compares finished during
                        # m0/m1, so it slots in here without stalling the PE
                        pending_l4[0]()
                        pending_l4[0] = None

                norm_binarize(lambda: (None, m_sb1), F1 // P, mm1, sink1)

                b2 = bpool.tile([P, len(kt3), RB], f8, tag="b2")

                def sink2(m, acc, m_sb):
                    nc.vector.tensor_tensor(b2[:, m, :], acc[:], m_sb[:], is_gt)

                norm_binarize(
                    lambda: mean_bcast_sum(
                        emit_dr(b1, w2_sb, len(kt2), (F2, NSUM2)), 1.0 / F2, 0.0
                    ),
                    F2 // P,
                    lambda m, acc: emit_dr(b1, w2_sb, len(kt2), (m * P, P))(acc[:]),
                    sink2,
                )

                b3 = bpool.tile([P, len(kt4), RB], f8, tag="b3")

                def sink3(m, acc, m_sb):
                    nc.vector.tensor_tensor(b3[:, m, :], acc[:], m_sb[:], is_gt)

                norm_binarize(
                    lambda: mean_bcast_sum(
                        emit_dr(b2, w3_sb, len(kt3), (F3, NSUM2)), 1.0 / F3, 0.0
                    ),
                    F3 // P,
                    lambda m, acc: emit_dr(b2, w3_sb, len(kt3), (m * P, P))(acc[:]),
                    sink3,
                )

                # ---- layer 4: plain DoubleRow matmul, no LN — deferred
                # into the next block's L1 stream so its compare deps clear
                def emit_l4(b3=b3, c0=c0, last=(blk == n_blocks - 1)):
                    acc4 = apool.tile([NC_OUT, RB], f32, tag="acc")
                    emit_dr(b3, w4_sb, len(kt4), (0, NC_OUT))(acc4[:])
                    out_sb = opool.tile([NC_OUT, RB], f32, tag="out")
                    nc.scalar.copy(out_sb[:], acc4[:])
                    if last:
                        # same queue as the copy: no cross-engine sem hop on
                        # the kernel's critical tail
                        nc.scalar.dma_start(out=out_d[:, c0 : c0 + RB], in_=out_sb[:])
                    else:
                        nc.sync.dma_start(out=out_d[:, c0 : c0 + RB], in_=out_sb[:])

                if blk + 1 < n_blocks:
                    pending_l4[0] = emit_l4
                else:
                    emit_l4()  # last block: emit now, only b3 sinks gate it

    nc.compile()
    return nc


def prep_host(x, w1, w2, w3, w4):
    """Returns per-input dict of full arrays."""
    w1b = (w1 > 0).astype(np.float32)
    top, bot = w1b[:D_IN], w1b[D_IN:]
    W1eff = top - bot
    c1 = bot.sum(0)
    W1rows = W1eff.sum(1)
    C1 = float(c1.sum())
    assert np.abs(W1rows).max() <= 256 and c1.max() <= 256

    def aug8(w, width):
        """fp8 layout: [binary cols | 3-way split of row-sums, twice | pad]."""
        wb = (w > 0).astype(np.float32)
        nf = wb.shape[1]
        rows = wb.sum(1)
        assert rows.max() <= 3 * 16, rows.max()
        out = np.zeros((wb.shape[0], width), np.float32)
        out[:, :nf] = wb
        rem = rows
        for i in range(NSUM):
            c = np.minimum(rem, 16.0)
            out[:, nf + i] = c
            out[:, nf + NSUM + i] = c
            rem = rem - c
        return out.astype(fp8)

    w2m, w3m = aug8(w2, W2PAD), aug8(w3, W3PAD)
    w4m = np.zeros((F3, W4PAD), np.float32)
    w4m[:, :NC_OUT] = (w4 > 0).astype(np.float32)
    w4m = w4m.astype(fp8)

    xT = np.ascontiguousarray(x.T).astype(np.float32)  # [784, B]
    B = x.shape[0]
    hi = xT.astype(fp16)
    resid = xT - hi.astype(np.float32)
    # features 0..767: fp8 lo chunk (resid*2^15 in [-8, 8], err <= 2^-17).
    # 2^-15 isn't fp8-representable on the weight side, so the WHOLE layer-1
    # accumulation runs at 2^15 scale: hi rows are hi*2^15 (exact exponent
    # shift in fp16), weights stay +-1, and m1 is scaled to match.
    xl = (resid[:KL] * LO_SCALE).astype(fp8)  # [768, B]
    # features 768..783: fp16 lo rides in the hi-chunk pad (same 2^15 scale)
    ride = (resid[KL:] * LO_SCALE).astype(fp16)  # [16, B], in [-8, 8]

    # hi chunk rows: [hi*2^15 (784) | 2^15 | rides(16) | zero pad to 896]
    xh = np.zeros((KH, B), fp16)
    xh[:D_IN] = (hi.astype(np.float32) * LO_SCALE).astype(fp16)  # exact
    xh[D_IN] = LO_SCALE
    xh[D_IN + 1 : D_IN + 1 + NRIDE] = ride

    w1h = np.zeros((KH, F1), fp16)
    w1h[:D_IN] = W1eff.astype(fp16)
    w1h[D_IN] = c1.astype(fp16)
    w1h[D_IN + 1 : D_IN + 1 + NRIDE] = W1eff[KL:].astype(fp16)
    w1l = W1eff[:KL].astype(fp8)  # [768, 2048], +-1 exact

    # layer-1 row-mean: affine in x — constant-fold on host (float64 dot of
    # the same quantized operands the device sees), at the same 2^15 scale
    xq = hi.astype(np.float64)
    xq[:KL] += xl.astype(np.float64) / LO_SCALE
    xq[KL:] += ride.astype(np.float64) / LO_SCALE
    S1 = xq.T @ W1rows.astype(np.float64) + C1
    m1 = (S1 / F1 * LO_SCALE).astype(np.float32)[None, :]  # [1, B]

    return {
        "xh": xh,
        "xl": xl,
        "w1h": w1h,
        "w1l": w1l,
        "m1": m1,
        "w2m": w2m,
        "w3m": w3m,
        "w4m": w4m,
    }


def _fallback_numpy(x, w1, w2, w3, w4, ln1_scale, ln1_bias, ln2_scale, ln2_bias,
                    ln3_scale, ln3_bias):
    """General path (arbitrary LN scale/bias): full fp32 LN on host."""
    h = np.concatenate([x, 1.0 - x], 1).astype(np.float32)
    for w, s, b in ((w1, ln1_scale, ln1_bias), (w2, ln2_scale, ln2_bias),
                    (w3, ln3_scale, ln3_bias)):
        a = h @ (w > 0).astype(np.float32)
        m = a.mean(1, dtype=np.float32, keepdims=True)
        v = np.mean((a - m) ** 2, axis=1, dtype=np.float32, keepdims=True)
        z = (a - m) / np.sqrt(v + 1e-6) * s + b
        h = (z > 0).astype(np.float32)
    return h @ (w4 > 0).astype(np.float32)


_CACHE = {}


def kernel(x, w1, w2, w3, w4, ln1_scale, ln1_bias, ln2_scale, ln2_bias,
           ln3_scale, ln3_bias, _trace=False):
    x = np.asarray(x, np.float32)
    fast = (
        np.all(np.asarray(ln1_scale) == 1) and np.all(np.asarray(ln1_bias) == 0)
        and np.all(np.asarray(ln2_scale) == 1) and np.all(np.asarray(ln2_bias) == 0)
        and np.all(np.asarray(ln3_scale) == 1) and np.all(np.asarray(ln3_bias) == 0)
    )
    if not fast or x.shape[0] % (N_CORES * RB) != 0:
        return _fallback_numpy(
            x, np.asarray(w1), np.asarray(w2), np.asarray(w3), np.asarray(w4),
            np.asarray(ln1_scale), np.asarray(ln1_bias), np.asarray(ln2_scale),
            np.asarray(ln2_bias), np.asarray(ln3_scale), np.asarray(ln3_bias),
        ).astype(np.float32)

    from concourse.bass_utils import run_bass_kernel_spmd

    arrs = prep_host(
        x, np.asarray(w1), np.asarray(w2), np.asarray(w3), np.asarray(w4)
    )
    B = x.shape[0]
    R = B // N_CORES
    n_blocks = R // RB

    if n_blocks not in _CACHE:
        _CACHE[n_blocks] = build_bass(n_blocks)
    nc = _CACHE[n_blocks]

    def blockmajor(a, sl):
        # [K, R] -> [n_blocks, P, K//P, RB]: per-partition-contiguous blocks
        k = a.shape[0]
        return np.ascontiguousarray(
            a[:, sl].reshape(k // P, P, n_blocks, RB).transpose(2, 1, 0, 3)
        )

    def wmajor(w, j0, j1):
        # [K, F1] cols j0:j1 -> [P, K//P, j1-j0] partition-contiguous
        k = w.shape[0]
        return np.ascontiguousarray(
            w[:, j0:j1].reshape(k // P, P, j1 - j0).transpose(1, 0, 2)
        )

    WCH = (F1 - 128) // 4
    w1h0 = wmajor(arrs["w1h"], 0, 128)
    w1hb = np.stack([wmajor(arrs["w1h"], 128 + c * WCH, 128 + (c + 1) * WCH)
                     for c in range(4)])
    w1l0 = wmajor(arrs["w1l"], 0, 128)
    w1lb = np.stack([wmajor(arrs["w1l"], 128 + c * WCH, 128 + (c + 1) * WCH)
                     for c in range(4)])

    in_maps = []
    for c in range(N_CORES):
        sl = slice(c * R, (c + 1) * R)
        m = {
            "xh": blockmajor(arrs["xh"], sl),
            "xl": blockmajor(arrs["xl"], sl),
            "w1h0": w1h0,
            "w1hb": w1hb,
            "w1l0": w1l0,
            "w1lb": w1lb,
            "m1": np.ascontiguousarray(arrs["m1"][:, sl]),
            "w2m": arrs["w2m"],
            "w3m": arrs["w3m"],
            "w4m": arrs["w4m"],
        }
        in_maps.append(m)

    res = run_bass_kernel_spmd(
        nc, in_maps, core_ids=list(range(N_CORES)), trace=_trace
    )
    out = np.concatenate([res.results[c]["out"] for c in range(N_CORES)], axis=1)
    if _trace:
        kernel._last_result = res
    return np.ascontiguousarray(out.T).astype(np.float32)
